# revision 89
# baseline (speedup 1.0000x reference)
# Trainium2 Bass kernel for AttentionBlockWithWeights.
#
# Problem (per batch element n of N=32):
#   qn/kn/vn = LayerNorm over full (C=64,H=32,W=32) of query/key/value
#   q/k/v    = 1x1-conv channel mixing (64x64) + bias, viewed as (C, HW=1024)
#   S        = k^T q / sqrt(C)            (HW, HW)
#   attn     = softmax(S, axis=0)         (softmax over i for each column j)
#   out      = v @ attn                   (C, HW)
#   x        = (qn + out).reshape(C*HW)
# Outputs: x (N, 65536) and attn (N, 1024, 1024).
#
# Sharding: pure data parallel, 4 batch elements per core across 8 cores.
# On-core layout: two batch elements ("a pair") stacked on the 128 SBUF
# partitions for LN + projections (block-diagonal 128x128 weights); the
# attention phase runs per batch element with i in chunks of 128 partitions.
# Weight tensors are pre-arranged on the host (block-diag / stacked / row-
# broadcast) so every DMA is a contiguous load.

import numpy as np

N_CORES = 8
NB = 4  # batch elements per core
C = 64
HW = 1024
EPS = 1e-5

_nc_cache = {}


def _build_bass(apply_affine: bool):
    from contextlib import ExitStack

    import concourse.bacc as bacc
    import concourse.mybir as mybir
    import concourse.tile as tile

    f32 = mybir.dt.float32
    f32r = mybir.dt.float32r
    AF = mybir.ActivationFunctionType
    OP = mybir.AluOpType

    # Bacc (not plain Bass): its compile pipeline splits multi-sem waits into
    # event-semaphore/nop instructions — TRN2 HW allows at most 1 wait per
    # instruction and walrus codegen rejects more.
    nc = bacc.Bacc(trn_type="TRN2")

    q_in = nc.dram_tensor("q_in", [NB * C, HW], f32, kind="ExternalInput")
    k_in = nc.dram_tensor("k_in", [NB * C, HW], f32, kind="ExternalInput")
    v_in = nc.dram_tensor("v_in", [NB * C, HW], f32, kind="ExternalInput")
    # host-prepped weights: block-diag lhsT (c+64m, o+64m) = w[o, c]
    # one packed constants tensor: [wq_bd | wk_bd | wv_bd | bq_p | bk_p | bv_bc]
    wts_d = nc.dram_tensor("wts_all", [128, 3 * 128 + 2 + C], f32, kind="ExternalInput")
    b2_d = nc.dram_tensor("b2_c", [2, 128], f32, kind="ExternalInput")
    ln_dram = {}
    if apply_affine:
        for i in (1, 2, 3):
            ln_dram[f"ln{i}_w"] = nc.dram_tensor(f"ln{i}_w", [128, HW], f32, kind="ExternalInput")
            ln_dram[f"ln{i}_b"] = nc.dram_tensor(f"ln{i}_b", [128, HW], f32, kind="ExternalInput")

    x_out = nc.dram_tensor("x_out", [NB * C, HW], f32, kind="ExternalOutput")
    a_out = nc.dram_tensor("a_out", [NB * HW, HW], f32, kind="ExternalOutput")

    import concourse.bass as bass

    with ExitStack() as ctx:
        tc = ctx.enter_context(tile.TileContext(nc))

        singles = ctx.enter_context(tc.tile_pool(name="singles", bufs=1))
        xp = ctx.enter_context(tc.tile_pool(name="xp", bufs=6))
        qnp = ctx.enter_context(tc.tile_pool(name="qnp", bufs=2))
        xnp = ctx.enter_context(tc.tile_pool(name="xnp", bufs=2))
        prjp = ctx.enter_context(tc.tile_pool(name="prjp", bufs=2))
        vtp = ctx.enter_context(tc.tile_pool(name="vtp", bufs=16))
        # affine path holds 3 MB of LN weight tiles -> shrink the E pool
        ep = ctx.enter_context(tc.tile_pool(name="ep", bufs=4 if apply_affine else 5))
        bcp = ctx.enter_context(tc.tile_pool(name="bcp", bufs=2))
        omp = ctx.enter_context(tc.tile_pool(name="omp", bufs=2))
        rcp = ctx.enter_context(tc.tile_pool(name="rcp", bufs=2))
        smp = ctx.enter_context(tc.tile_pool(name="smp", bufs=8))
        psp = ctx.enter_context(tc.tile_pool(name="psp", bufs=2, space="PSUM"))
        ps1 = ctx.enter_context(tc.tile_pool(name="ps1", bufs=4, space="PSUM"))

        # ---- constants built on-chip (no DMA) ----
        # blkones[p, m] = 1 if batch-half m owns partition p (for per-n stat sums)
        blkones = singles.tile([128, 2], f32)
        nc.vector.memset(blkones, 0.0)
        nc.vector.memset(blkones[0:64, 0:1], 1.0)
        nc.vector.memset(blkones[64:128, 1:2], 1.0)
        ones_sq = singles.tile([128, 128], f32)
        nc.vector.memset(ones_sq, 1.0)

        # hoist the Exp activation-table load (~1.3us) off the critical path
        warm_e = smp.tile([2, 1], f32, tag="warm_e")
        nc.scalar.activation(warm_e, blkones[0:2, 0:1], AF.Exp, scale=1.0)

        # ---- phase A: LayerNorm + projections, per-pair ----
        qn_tiles = []
        qn_lo_tiles = []
        qp_tiles = []
        kp_tiles = []
        vt_tiles = [[], []]
        PT = (("q", q_in, 1), ("k", k_in, 2), ("v", v_in, 3))

        # all input loads up front: they go first in the HWDGE FIFO (the LN
        # stats chain is the critical path), then the constants, then pair 1.
        x_tiles = [{}, {}]
        for pair in range(2):
            rows = slice(pair * 128, (pair + 1) * 128)
            for tname, dram, lni in PT:
                x_t = xp.tile([128, HW], f32, tag="x", name=f"x_{pair}{tname}")
                # two half-loads so bn_stats starts on the first half early
                nc.sync.dma_start(out=x_t[:, 0:512], in_=dram[rows, 0:512])
                nc.sync.dma_start(out=x_t[:, 512:1024], in_=dram[rows, 512:1024])
                x_tiles[pair][tname] = x_t
            if pair == 0:
                # constants: one packed load, queued behind pair 0's inputs
                wts = singles.tile([128, 3 * 128 + 2 + C], f32)
                nc.sync.dma_start(out=wts, in_=wts_d[:, :])
                B2 = singles.tile([2, 128], f32)
                nc.sync.dma_start(out=B2, in_=b2_d[:, :])

        wv_bdT = wts[:, 256:384]
        bias_pair = {"q": wts[:, 384:385], "k": wts[:, 385:386]}
        bv_bc = wts[:, 386:386 + C]
        # fp32r-rounded copies: producers of fp32r-matmul operands must round
        w_bd = {}
        for wi, name in ((0, "q"), (1, "k")):
            t = singles.tile([128, 128], f32, tag=f"w{name}r", name=f"w{name}r")
            nc.vector.tensor_scalar_mul(
                t.bitcast(f32r), wts[:, wi * 128:(wi + 1) * 128], 1.0
            )
            w_bd[name] = t
        ones_r = singles.tile([128, 128], f32)
        nc.vector.tensor_scalar_mul(ones_r.bitcast(f32r), ones_sq, 1.0)

        ln_pair = {}
        if apply_affine:
            for key, h in ln_dram.items():
                t = singles.tile([128, HW], f32, tag=f"t{key}", name=f"t{key}")
                nc.sync.dma_start(out=t, in_=h[:, :])
                ln_pair[key] = t

        def emit_A_stats(pair, subset):
            # per-n stat rows: column t of [2, nt] = tensor t's two halves.
            # q/k are a separate chain from v: only they gate the S matmuls.
            nt = len(subset)
            sfx = f"{pair}" + "".join(t[0] for t in subset)
            means_all = smp.tile([2, nt], f32, tag=f"mns{sfx}", name=f"mns_{sfx}")
            u_all = smp.tile([2, nt], f32, tag=f"u{sfx}", name=f"u_{sfx}")

            for t_i, (tname, dram, lni) in enumerate(subset):
                x_t = x_tiles[pair][tname]
                st6 = smp.tile([128, 2, 6], f32, tag="st6", name=f"st6_{pair}{tname}")
                nc.vector.bn_stats(out=st6[:, 0, :], in_=x_t[:, 0:512])
                nc.vector.bn_stats(out=st6[:, 1, :], in_=x_t[:, 512:1024])
                mv = smp.tile([128, 2], f32, tag="mv", name=f"mv_{pair}{tname}")
                nc.vector.bn_aggr(out=mv, in_=st6)

                # fold E[x^2] into mv in place: mv[:,1] = mean^2 + var
                nc.vector.scalar_tensor_tensor(
                    mv[:, 1:2], mv[:, 0:1], mv[:, 0:1], mv[:, 1:2],
                    op0=OP.mult, op1=OP.add,
                )

                # cross-partition: per-n sums of [mean, E[x^2]]
                stat_ps = ps1.tile([2, 2], f32, tag="onebank", name=f"sps_{pair}{tname}")
                nc.tensor.matmul(stat_ps, lhsT=blkones, rhs=mv, start=True, stop=True)
                mcol = means_all[:, t_i:t_i + 1]
                nc.vector.tensor_scalar_mul(mcol, stat_ps[:, 0:1], 1.0 / 64.0)
                ex2 = smp.tile([2, 1], f32, tag="ex2", name=f"ex2_{pair}{tname}")
                nc.vector.tensor_scalar_mul(ex2, stat_ps[:, 1:2], 1.0 / 64.0)
                # u_all holds NEGATED variance: mean^2 - E[x^2]
                nc.vector.scalar_tensor_tensor(
                    u_all[:, t_i:t_i + 1], mcol, mcol, ex2,
                    op0=OP.mult, op1=OP.subtract,
                )
            return means_all, u_all

        def emit_A_rest(pair, subset, means_all, u_all):
            nt = len(subset)
            sfx = f"{pair}" + "".join(t[0] for t in subset)
            # rstd for the pair's 3 tensors at once, pure DVE:
            # y = (var+eps)^-1/2 via y0 = 1/(var+eps) (var ~= 1 so y0 is close)
            # + Newton y <- y*(1.5 - 0.5*u*y^2). No ACT table funcs -> the Exp
            # set stays resident for the attention phase.
            # u_all = -(var); ue = var + eps = -u_all + eps
            ue = smp.tile([2, nt], f32, tag=f"ue{sfx}", name=f"ue_{sfx}")
            nc.vector.tensor_scalar(ue, u_all, -1.0, EPS, op0=OP.mult, op1=OP.add)
            y = smp.tile([2, nt], f32, tag=f"y{sfx}", name=f"y_{sfx}")
            nc.vector.reciprocal(y, ue)
            ta = smp.tile([2, nt], f32, tag=f"ta{sfx}", name=f"ta_{sfx}")
            for _ in range(4):
                nc.vector.tensor_mul(ta, y, y)
                nc.vector.tensor_mul(ta, ta, ue)
                nc.vector.tensor_scalar(ta, ta, -0.5, 1.5, op0=OP.mult, op1=OP.add)
                nc.vector.tensor_mul(y, y, ta)
            nmr = smp.tile([2, nt], f32, tag=f"nmr{sfx}", name=f"nmr_{sfx}")
            nc.vector.tensor_mul(nmr, means_all, y)
            nc.vector.tensor_scalar_mul(nmr, nmr, -1.0)

            # broadcast per-n [rstd, -mean*rstd] to per-partition scalars
            scl_ps_a = ps1.tile([128, nt], f32, tag="onebank", name=f"spa_{sfx}")
            nc.tensor.matmul(scl_ps_a, lhsT=B2, rhs=y, start=True, stop=True)
            scl_ps_b = ps1.tile([128, nt], f32, tag="onebank", name=f"spb_{sfx}")
            nc.tensor.matmul(scl_ps_b, lhsT=B2, rhs=nmr, start=True, stop=True)
            scl_a = smp.tile([128, nt], f32, tag=f"scl_a{sfx}", name=f"sa_{sfx}")
            nc.vector.tensor_copy(scl_a, scl_ps_a)
            scl_b = smp.tile([128, nt], f32, tag=f"scl_b{sfx}", name=f"sb_{sfx}")
            nc.vector.tensor_copy(scl_b, scl_ps_b)

            # LN apply + projections / v^T
            for t_i, (tname, dram, lni) in enumerate(subset):
                x_t = x_tiles[pair][tname]
                pool = qnp if tname == "q" else xnp
                xn = pool.tile([128, HW], f32, tag=("qn" if tname == "q" else "xn"),
                               name=f"xn_{pair}{tname}")
                # normalize on DVE (tensor_scalar 2x fp32 mode; DVE is idle
                # right after the stats chain while ACT gates the exp stream):
                # xn = x*rstd + (-mean*rstd), q/k rounded for the fp32r proj
                xn_out = xn if tname == "v" else xn.bitcast(f32r)
                nc.vector.tensor_scalar(
                    xn_out, x_t,
                    scl_a[:, t_i:t_i + 1], scl_b[:, t_i:t_i + 1],
                    op0=OP.mult, op1=OP.add,
                )
                if apply_affine:
                    nc.vector.tensor_mul(xn, xn, ln_pair[f"ln{lni}_w"])
                    nc.vector.tensor_add(xn, xn, ln_pair[f"ln{lni}_b"])
                if tname == "q":
                    qn_tiles.append(xn)
                    # pre-shift qn's half-1 down to partitions 0..63 now, while
                    # the DMA engines are idle — the odd-n residual add then
                    # needs no mid-stream shuffle
                    qn_lo = qnp.tile([64, HW], f32, tag="qnlo", name=f"qnlo_{pair}")
                    nc.sync.dma_start(out=qn_lo, in_=xn[64:128, :])
                    qn_lo_tiles.append(qn_lo)

                if tname in ("q", "k"):
                    pr_ps = psp.tile([128, HW], f32, tag="mm1024",
                                     name=f"prps_{pair}{tname}")
                    nc.tensor.matmul(
                        pr_ps[:, 0:512], lhsT=w_bd[tname].bitcast(f32r),
                        rhs=xn[:, 0:512].bitcast(f32r),
                        start=True, stop=True,
                    )
                    nc.tensor.matmul(
                        pr_ps[:, 512:1024], lhsT=w_bd[tname].bitcast(f32r),
                        rhs=xn[:, 512:1024].bitcast(f32r),
                        start=True, stop=True,
                    )
                    pp = prjp.tile([128, HW], f32, tag=f"{tname}p",
                                   name=f"pp_{pair}{tname}")
                    # write rounded to fp32r: consumed by the fp32r S matmuls
                    nc.scalar.activation(
                        pp.bitcast(f32r), pr_ps, AF.Identity,
                        bias=bias_pair[tname], scale=1.0,
                    )
                    (qp_tiles if tname == "q" else kp_tiles).append(pp)
                else:
                    # v^T chunks: out[i, (m, c)] = v_m[c, i] for both halves m
                    for ic in range(8):
                        vt_ps = ps1.tile([128, 128], f32, tag="onebank",
                                         name=f"vtps_{pair}_{ic}")
                        nc.tensor.matmul(
                            vt_ps, lhsT=xn[:, ic * 128:(ic + 1) * 128], rhs=wv_bdT,
                            start=True, stop=True,
                        )
                        vt = vtp.tile([128, 2, 65], f32, tag="vt",
                                      name=f"vt_{pair}_{ic}")
                        nc.vector.tensor_add(
                            vt[:, 0, 0:64].bitcast(f32r), vt_ps[:, 0:64], bv_bc
                        )
                        nc.vector.tensor_add(
                            vt[:, 1, 0:64].bitcast(f32r), vt_ps[:, 64:128], bv_bc
                        )
                        # ones column for the denominator row of v@E (memset
                        # can't write f32r-rounded; tensor_scalar can)
                        nc.vector.tensor_scalar_mul(
                            vt[:, :, 64:65].bitcast(f32r), ones_sq[:, 0:2], 1.0
                        )
                        vt_tiles[pair].append(vt)

        # ---- phase B: attention per batch element ----
        # E tiles hold 4 i-chunks (128 x 4096) so each attn store is one 2 MiB
        # contiguous-row DMA.
        def emit_B(n):
            pair, half = n // 2, n % 2
            hs = slice(half * 64, half * 64 + 64)
            qn = qn_tiles[pair]
            qp = qp_tiles[pair]
            kp = kp_tiles[pair]

            e_tiles = []
            vu = [None, None]
            for g in range(2):
                e2 = ep.tile([128, 4 * HW], f32, tag="e", name=f"e_{n}_{g}")
                e_tiles.append(e2)
                for b in range(4):
                    ic = g * 4 + b
                    s_ps = psp.tile([128, HW], f32, tag="mm1024", name=f"sps_{n}_{ic}")
                    # float32r: single-pass fp32 matmul (1 cycle/row vs 4 for
                    # exact fp32) — these two groups are ~80% of PE time.
                    lhsT = kp[hs, ic * 128:(ic + 1) * 128].bitcast(f32r)
                    nc.tensor.matmul(
                        s_ps[:, 0:512], lhsT=lhsT, rhs=qp[hs, 0:512].bitcast(f32r),
                        start=True, stop=True,
                    )
                    nc.tensor.matmul(
                        s_ps[:, 512:1024], lhsT=lhsT,
                        rhs=qp[hs, 512:1024].bitcast(f32r),
                        start=True, stop=True,
                    )
                    # E = exp(S / sqrt(C));  sqrt(C)=8 -> scale=0.125.
                    # Written fp32r-rounded: consumed by the fp32r v@E matmul.
                    ecol = e2[:, b * HW:(b + 1) * HW]
                    nc.scalar.activation(ecol.bitcast(f32r), s_ps, AF.Exp, scale=0.125)

                    if ic == 0:
                        vu[0] = ps1.tile([65, 512], f32, tag="onebank", name=f"vu0_{n}")
                        vu[1] = ps1.tile([65, 512], f32, tag="onebank", name=f"vu1_{n}")
                    vt = vt_tiles[pair][ic]
                    nc.tensor.matmul(
                        vu[0], lhsT=vt[:, half, :].bitcast(f32r),
                        rhs=e2[:, b * HW:b * HW + 512].bitcast(f32r),
                        start=(ic == 0), stop=(ic == 7),
                    )
                    nc.tensor.matmul(
                        vu[1], lhsT=vt[:, half, :].bitcast(f32r),
                        rhs=e2[:, b * HW + 512:(b + 1) * HW].bitcast(f32r),
                        start=(ic == 0), stop=(ic == 7),
                    )

            # vu rows 0..63 = v @ E, row 64 = column sums of E (softmax denom)
            rc = rcp.tile([128, HW], f32, tag="rc", name=f"rc_{n}")
            with nc.allow_low_precision("fp32r-rounded recip feeds fp32r bc matmul"):
                nc.vector.reciprocal(rc[64:65, 0:512].bitcast(f32r), vu[0][64:65, :])
                nc.vector.reciprocal(rc[64:65, 512:1024].bitcast(f32r), vu[1][64:65, :])
            bc_ps = psp.tile([128, HW], f32, tag="mm1024", name=f"bcps_{n}")
            nc.tensor.matmul(
                bc_ps[:, 0:512], lhsT=ones_r[64:65, :].bitcast(f32r),
                rhs=rc[64:65, 0:512].bitcast(f32r),
                start=True, stop=True,
            )
            nc.tensor.matmul(
                bc_ps[:, 512:1024], lhsT=ones_r[64:65, :].bitcast(f32r),
                rhs=rc[64:65, 512:1024].bitcast(f32r),
                start=True, stop=True,
            )
            bc = bcp.tile([128, HW], f32, tag="bc", name=f"bc_{n}")
            nc.vector.tensor_copy(bc, bc_ps)  # DVE: ACT is exp-saturated

            # x = qn + (v @ E) * (1/denom)
            om = omp.tile([128, HW], f32, tag="om", name=f"om_{n}")
            nc.vector.tensor_mul(om[0:64, 0:512], vu[0][0:64, :], bc[0:64, 0:512])
            nc.vector.tensor_mul(om[0:64, 512:1024], vu[1][0:64, :], bc[0:64, 512:1024])
            qn_res = qn[0:64, :] if half == 0 else qn_lo_tiles[pair]
            nc.vector.tensor_add(om[0:64, :], om[0:64, :], qn_res)
            nc.sync.dma_start(out=x_out[n * 64:(n + 1) * 64, :], in_=om[0:64, :])

            # attn = E * (1/denom); split the multiply between DVE and GpSimd
            # (Pool is otherwise idle). The DVE half reads the broadcast
            # straight from PSUM (skips waiting on the ACT copy); GpSimd can't
            # touch PSUM so it reads the SBUF copy. One 1 MiB store per tile.
            for g in range(2):
                e2 = e_tiles[g]
                dve_share = 1 if n < 3 else 2
                for b in range(4):
                    eng = nc.vector if b < dve_share else nc.gpsimd
                    eng.tensor_mul(
                        e2[:, b * HW:(b + 1) * HW].bitcast(f32r),
                        e2[:, b * HW:(b + 1) * HW], bc,
                    )
                for hh in range(2):
                    base = (n * HW + g * 512 + hh * 256) * HW
                    out_ap = bass.AP(
                        tensor=a_out[:, :].tensor, offset=base,
                        ap=[[HW, 128], [128 * HW, 2], [1, HW]],
                    )
                    nc.sync.dma_start(
                        out=out_ap,
                        in_=e2[:, hh * 2 * HW:(hh + 1) * 2 * HW].rearrange(
                            "p (b f) -> p b f", f=HW
                        ),
                    )

        # Emission order == scheduler priority: finish pair 0's attention
        # (stores start flowing early); pair 1's LN stats fill DVE gaps during
        # pair 0's matmul/exp-heavy stretch.
        m0, u0 = emit_A_stats(0, PT)
        emit_A_rest(0, PT, m0, u0)
        emit_B(0)
        m1, u1 = emit_A_stats(1, PT)
        emit_B(1)
        emit_A_rest(1, PT, m1, u1)
        emit_B(2)
        emit_B(3)

    # Bacc defers register allocation etc. to finalize(); the pjrt exec path
    # requires an already-finalized module.
    nc.finalize()
    return nc


def _get_nc(apply_affine: bool):
    if apply_affine not in _nc_cache:
        _nc_cache[apply_affine] = _build_bass(apply_affine)
    return _nc_cache[apply_affine]


def _host_weights(inputs):
    f = lambda a: np.ascontiguousarray(np.asarray(a), dtype=np.float32)
    cols = []
    for name in ("q", "k", "v"):
        w = f(inputs[f"w{name}"])  # [o, c]
        bd = np.zeros((128, 128), np.float32)
        bd[0:64, 0:64] = w.T
        bd[64:128, 64:128] = w.T
        cols.append(bd)
    for name in ("q", "k"):
        b = f(inputs[f"b{name}"])
        cols.append(np.concatenate([b, b]).reshape(128, 1))
    bv = f(inputs["bv"])
    cols.append(np.broadcast_to(bv[None, :], (128, C)))
    b2 = np.zeros((2, 128), np.float32)
    b2[0, 0:64] = 1.0
    b2[1, 64:128] = 1.0
    return {
        "wts_all": np.ascontiguousarray(np.concatenate(cols, axis=1)),
        "b2_c": b2,
    }


def _prep_in_maps(inputs):
    f = lambda a: np.ascontiguousarray(np.asarray(a), dtype=np.float32)
    query = f(inputs["query"]).reshape(32, C, HW)
    key = f(inputs["key"]).reshape(32, C, HW)
    value = f(inputs["value"]).reshape(32, C, HW)
    ln = {k: f(inputs[k]).reshape(C, HW) for k in
          ("ln1_w", "ln1_b", "ln2_w", "ln2_b", "ln3_w", "ln3_b")}
    apply_affine = not (
        all(np.all(ln[f"ln{i}_w"] == 1.0) for i in (1, 2, 3))
        and all(np.all(ln[f"ln{i}_b"] == 0.0) for i in (1, 2, 3))
    )
    base = dict(_host_weights(inputs))
    if apply_affine:
        for k in ln:
            base[k] = np.vstack([ln[k], ln[k]]).copy()  # [128, HW] pair tile
    in_maps = []
    for c in range(N_CORES):
        sl = slice(c * NB, (c + 1) * NB)
        m = dict(base)
        m["q_in"] = np.ascontiguousarray(query[sl].reshape(NB * C, HW))
        m["k_in"] = np.ascontiguousarray(key[sl].reshape(NB * C, HW))
        m["v_in"] = np.ascontiguousarray(value[sl].reshape(NB * C, HW))
        in_maps.append(m)
    return in_maps, apply_affine


def _assemble(results):
    x = np.empty((32, C * HW), dtype=np.float32)
    attn = np.empty((32, HW, HW), dtype=np.float32)
    for c, res in enumerate(results):
        x[c * NB:(c + 1) * NB] = res["x_out"].reshape(NB, C * HW)
        attn[c * NB:(c + 1) * NB] = res["a_out"].reshape(NB, HW, HW)
    return x, attn


def _run(inputs, **spmd_kwargs):
    from concourse.bass_utils import run_bass_kernel_spmd

    in_maps, apply_affine = _prep_in_maps(inputs)
    nc = _get_nc(apply_affine)
    br = run_bass_kernel_spmd(nc, in_maps, list(range(N_CORES)), **spmd_kwargs)
    return br


def kernel(**inputs):
    br = _run(inputs)
    return _assemble(br.results)
